# revision 8
# baseline (speedup 1.0000x reference)
"""Trainium2 Bass kernel for 5-sweep Jacobi iteration (4th-order 2D Poisson).

Problem: B=16 samples of [1024,1024] f32; per-sample cross stencil from dx;
5 Jacobi sweeps; 2-wide boundary frame kept fixed at the initial guess.

Sharding: data-parallel over batch, 2 samples per core, 8 cores. Whole
working set (2 x (guess + scaled rhs) = 16 MB) stays resident in SBUF.

Per sweep the stencil is evaluated on the TensorEngine as PSUM-accumulated
matmuls per [128, 512] output unit:
  - banded lhsT   (H-direction taps within the 128-row block)
  - K=2 halo mms  (H taps crossing the block boundary)
  - scaled-identity lhsT with col-shifted rhs APs (W-direction taps)
The VectorEngine then evacuates: new = R + psum (R = dinv * rhs, prescaled
once). Boundary rows/cols are never written, so they keep the initial-guess
values both ping-pong buffers are loaded with.
"""

import sys

sys.path.insert(0, "/opt/trn_rl_repo")

import numpy as np

N_CORES = 8
B, H, W = 16, 1024, 1024
SPC = B // N_CORES  # samples per core
P = 128
NBLK = H // P  # 8 row-blocks
FREE = NBLK * W  # 8192
GRD = 2  # guard cols each side of the g buffers
N_ITER = 5
NHALF = 2  # W halves of 512 (PSUM bank limit for fp32)
MM_DT = "float32r"  # matmul input dtype: float32r | float32 | bfloat16

_CACHE = {}


def _host_coeffs(dx):
    """Per-sample stencil scalars in float64. dx: [B, 2]."""
    a = (1.0 / dx.astype(np.float64)) ** 2
    a0, a1 = a[:, 0], a[:, 1]
    dinv = 1.0 / (-2.5 * (a0 + a1))
    e1 = dinv * a0 * (4.0 / 3.0)
    e2 = dinv * a0 * (-1.0 / 12.0)
    f1 = dinv * a1 * (4.0 / 3.0)
    f2 = dinv * a1 * (-1.0 / 12.0)
    return dinv, e1, e2, f1, f2


def _host_mats(dx):
    """Build [B, 128, 640] lhsT matrices: [Bc | Htop | Hbot | FI1 | FI2].

    All entries are the NEGATED dinv-scaled tap coefficients so that
    psum = -dinv*cr and new = dinv*rhs + psum.
    """
    dinv, e1, e2, f1, f2 = _host_coeffs(dx)
    nb = dx.shape[0]
    mats = np.zeros((nb, P, 5 * P), np.float64)
    idx = np.arange(P)
    for b in range(nb):
        bc = mats[b, :, 0:128]
        for off, v in ((1, -e1[b]), (-1, -e1[b]), (2, -e2[b]), (-2, -e2[b])):
            kk = idx[(idx + off >= 0) & (idx + off < P)]
            bc[kk, kk + off] = v
        ht = mats[b, :, 128:256]
        ht[126, 0] = -e2[b]
        ht[127, 0] = -e1[b]
        ht[127, 1] = -e2[b]
        hb = mats[b, :, 256:384]
        hb[0, 126] = -e2[b]
        hb[0, 127] = -e1[b]
        hb[1, 127] = -e2[b]
        mats[b, :, 384:512][idx, idx] = -f1[b]
        mats[b, :, 512:640][idx, idx] = -f2[b]
    return mats.astype(np.float32), dinv.astype(np.float32)


def _build_nc():
    import concourse.bacc as bacc
    import concourse.tile as tile
    from concourse import mybir

    f32 = mybir.dt.float32
    mm_dt = getattr(mybir.dt, MM_DT)
    nc = bacc.Bacc(
        "TRN2",
        target_bir_lowering=False,
        debug=False,
        enable_asserts=False,
        num_devices=N_CORES,
    )
    g_d = nc.dram_tensor("g", [SPC, P, FREE + 2 * GRD], mm_dt, kind="ExternalInput").ap()
    r_d = nc.dram_tensor("r", [SPC, P, FREE], f32, kind="ExternalInput").ap()
    m_d = nc.dram_tensor("m", [SPC, P, 5 * P], mm_dt, kind="ExternalInput").ap()
    c_d = nc.dram_tensor("c", [SPC, P, 1], f32, kind="ExternalInput").ap()
    o_d = nc.dram_tensor("o", [SPC, P, FREE], f32, kind="ExternalOutput").ap()

    with tile.TileContext(nc) as tc:
        with (
            tc.tile_pool(name="state", bufs=1) as state,
            tc.tile_pool(name="psum", bufs=8, space="PSUM") as pp,
        ):
            gb = [
                [state.tile([P, FREE + 2 * GRD], mm_dt, name=f"g{s}_{i}", tag=f"g{s}_{i}") for i in range(2)]
                for s in range(SPC)
            ]
            rb = [state.tile([P, FREE], f32, name=f"r{s}", tag=f"r{s}") for s in range(SPC)]
            mt = [state.tile([P, 5 * P], mm_dt, name=f"m{s}", tag=f"m{s}") for s in range(SPC)]
            cf = [state.tile([P, 1], f32, name=f"c{s}", tag=f"c{s}") for s in range(SPC)]

            for s in range(SPC):
                for i in range(2):
                    nc.sync.dma_start(gb[s][i][:], g_d[s])
                nc.sync.dma_start(rb[s][:], r_d[s])
                nc.sync.dma_start(mt[s][:], m_d[s])
                nc.sync.dma_start(cf[s][:], c_d[s])
                # R = dinv * rhs, in place
                nc.vector.tensor_scalar_mul(rb[s][:], rb[s][:], cf[s][:, 0:1])

            for it in range(N_ITER):
                for s in range(SPC):
                    cur = gb[s][it % 2]
                    nxt = gb[s][(it + 1) % 2]
                    for k in range(NBLK):
                        for xh in range(NHALF):
                            ps = pp.tile([P, 512], f32, name="ps", tag="ps")
                            cb = GRD + W * k + 512 * xh
                            mms = []
                            # H main: banded Bc
                            mms.append((mt[s][:, 0:128], cur[:, cb : cb + 512], None))
                            # W shifts: FI1 (+-1), FI2 (+-2)
                            for mcol, d in ((384, -1), (384, 1), (512, -2), (512, 2)):
                                mms.append(
                                    (
                                        mt[s][:, mcol : mcol + 128],
                                        cur[:, cb + d : cb + d + 512],
                                        None,
                                    )
                                )
                            # halo mms: full K=128 with zero-padded lhsT rows
                            # (K-subtiling via tile_position crashes the device)
                            if k > 0:  # prev block rows 126,127 -> out rows 0,1
                                mms.append(
                                    (mt[s][:, 128:256], cur[:, cb - W : cb - W + 512], None)
                                )
                            if k < NBLK - 1:  # next block rows 0,1 -> out 126,127
                                mms.append(
                                    (mt[s][:, 256:384], cur[:, cb + W : cb + W + 512], None)
                                )
                            for j, (lhsT, rhs, tpos) in enumerate(mms):
                                nc.tensor.matmul(
                                    ps[:, :],
                                    lhsT,
                                    rhs,
                                    start=(j == 0),
                                    stop=(j == len(mms) - 1),
                                    skip_group_check=True,
                                    tile_position=tpos,
                                )
                            # evacuate: new = R + psum. Partition bases must be
                            # quadrant-aligned, so block 7 stops at row 126 and
                            # block 0 evacuates rows 0,1 too (restored below).
                            p1 = 126 if k == NBLK - 1 else 128
                            n0 = 2 if xh == 0 else 0
                            n1 = 510 if xh == NHALF - 1 else 512
                            rbase = W * k + 512 * xh
                            nc.vector.tensor_add(
                                nxt[0:p1, cb + n0 : cb + n1],
                                ps[0:p1, n0:n1],
                                rb[s][0:p1, rbase + n0 : rbase + n1],
                            )
                        # restore boundary rows 0,1 clobbered by the full evac
                        if k == 0:
                            nc.scalar.copy(
                                nxt[0:2, GRD : GRD + W], cur[0:2, GRD : GRD + W]
                            )

            for s in range(SPC):
                final = gb[s][N_ITER % 2]
                nc.sync.dma_start(o_d[s], final[:, GRD : GRD + FREE].bitcast(f32))

    nc.compile()
    return nc


def _get_nc():
    if "nc" not in _CACHE:
        _CACHE["nc"] = _build_nc()
    return _CACHE["nc"]


def _round_f32r(x):
    """Round fp32 to float32r precision (11 explicit mantissa bits, RNE)."""
    if MM_DT != "float32r":
        return x
    b = np.ascontiguousarray(x, dtype=np.float32).view(np.uint32)
    drop = 12
    lsb = (b >> drop) & np.uint32(1)
    b = (b + np.uint32((1 << (drop - 1)) - 1) + lsb) & np.uint32(~((1 << drop) - 1) & 0xFFFFFFFF)
    return b.view(np.float32)


def _to_block(x):
    """[B, H, W] -> [B, P, FREE]: out[b, p, k*W + x] = in[b, 128k+p, x]."""
    nb = x.shape[0]
    return np.ascontiguousarray(
        x.reshape(nb, NBLK, P, W).transpose(0, 2, 1, 3).reshape(nb, P, FREE)
    )


def _from_block(x):
    nb = x.shape[0]
    return np.ascontiguousarray(
        x.reshape(nb, P, NBLK, W).transpose(0, 2, 1, 3).reshape(nb, H, W)
    )


def kernel(current_guess, rhses, dx):
    from concourse.bass_utils import run_bass_kernel_spmd

    g = _round_f32r(_to_block(np.ascontiguousarray(current_guess[:, 0], dtype=np.float32)))
    gpad = np.zeros((B, P, FREE + 2 * GRD), np.float32)
    gpad[:, :, GRD : GRD + FREE] = g
    g = gpad
    r = _to_block(np.ascontiguousarray(rhses[:, 0], dtype=np.float32))
    mats, dinv = _host_mats(dx)
    mats = _round_f32r(mats)
    coef = np.ascontiguousarray(
        np.broadcast_to(dinv[:, None, None], (B, P, 1)), dtype=np.float32
    )

    nc = _get_nc()
    in_maps = []
    for c in range(N_CORES):
        sl = slice(c * SPC, (c + 1) * SPC)
        in_maps.append(
            {
                "g": np.ascontiguousarray(g[sl]),
                "r": np.ascontiguousarray(r[sl]),
                "m": np.ascontiguousarray(mats[sl]),
                "c": np.ascontiguousarray(coef[sl]),
            }
        )
    res = run_bass_kernel_spmd(nc, in_maps, core_ids=list(range(N_CORES)))
    _CACHE["last_results"] = res
    ob = np.concatenate([res.results[c]["o"] for c in range(N_CORES)], axis=0)
    return _from_block(ob)[:, None].astype(np.float32)
